# revision 7
# baseline (speedup 1.0000x reference)
"""Trainium2 Bass kernel for nn_AutoencoderRNN_VAE_v2 (LSTM VAE autoencoder).

Data-parallel over the 1920 effective windows: 240 windows per NeuronCore.
All activations live feature-major ([feature_chunk<=128, batch] "transposed"
layout) so the LSTM recurrence needs no transposes. Matmuls run in float32r
(fp32 HIGH single-pass mode, ~1e-4 rounding, 1 cyc/row) with fp32 PSUM
accumulation.

Self-contained: hardcodes all shapes; builds + compiles the SPMD program on
first call and runs it on cores 0-7.
"""
import os
import sys
import types

sys.path.insert(0, "/opt/trn_rl_repo")
import numpy as np

B, T, Wn, F, H, Z, M, C3D = 128, 750, 50, 25, 512, 256, 512, 58
NCORE = 8
b = 240            # windows per core
b2 = 2 * b         # 480, merged half-tile free width
TOK = Wn * b       # 12000 tokens per core

_cache = {}


def _install_ntff_hook():
    try:
        import antenv
        from trn_agent_boot.trn_boot import _ntff_profile_via_ctypes
        hook = _ntff_profile_via_ctypes("/opt/axon/libaxon_pjrt.so")
        mod = types.ModuleType("antenv.axon_hooks")
        mod.get_axon_ntff_profile_hook = lambda: hook
        mod.set_axon_ntff_profile_hook = lambda h: None
        sys.modules["antenv.axon_hooks"] = mod
        antenv.axon_hooks = mod
    except Exception:
        pass


def _patch_tile_drain():
    """Stock kernel-tail drain carries one wait per live semaphore (11+),
    overflowing walrus's 1-wait-per-instruction budget. Split it."""
    import concourse.tile as tile
    from concourse.vector_clock import ScopedClock, VectorClock

    def _split_drain_and_barrier(self, tick_clock, wait_clock):
        vec = list(tick_clock.global_clock)
        for i, t in enumerate(vec):
            if t <= 0:
                continue
            mv = [0] * len(vec)
            mv[i] = t
            d = self.nc.sync.drain()
            wait_clock.add_sem_waits(d.ins, ScopedClock({None: VectorClock(mv)}))
        self.nc.all_engine_barrier()
        assert self.sems is not None
        popped = self.nc._tile_sem_poison_stack.pop()
        assert popped is self._sem_poison
        self.nc.clear_and_free_semaphores(list(self.sems.allocated().values()))
        self.nc.all_engine_barrier()

    tile.TileContext._drain_and_barrier = _split_drain_and_barrier


def _split_multiwait(nc):
    """walrus (this toolchain) accepts only ONE sync wait per TPB instruction.
    Hoist extra waits onto injected same-engine NOPs just before the
    offending instruction."""
    from concourse.mybir import SyncInfo

    def make_nop(engine):
        ins = nc.engines[engine].nop(nofuse=True).ins
        cur = nc.cur_bb.bb.instructions
        assert cur[-1] is ins
        cur.pop()
        return ins

    n_split = 0
    for f in nc.m.functions:
        for bb in f.blocks:
            new = []
            for inst in bb.instructions:
                si = getattr(inst, "sync_info", None)
                waits = list(si.on_wait) if si and si.on_wait else []
                if len(waits) > 1 and getattr(inst, "engine", None) is not None:
                    for w in waits[:-1]:
                        nop = make_nop(inst.engine)
                        nop.sync_info = SyncInfo(on_wait=[w], on_update=[])
                        new.append(nop)
                    inst.sync_info = SyncInfo(
                        on_wait=[waits[-1]], on_update=list(si.on_update or [])
                    )
                    n_split += 1
                new.append(inst)
            bb.instructions[:] = new
    return n_split


def _build_program():
    import concourse.bass as bass
    import concourse.mybir as mybir
    import concourse.tile as tile

    _patch_tile_drain()

    dt = mybir.dt
    r = dt.float32r
    f32 = dt.float32
    AF = mybir.ActivationFunctionType

    nc = bass.Bass(trn_type="TRN2", target_bir_lowering=False, debug=False)

    def din(name, shape, dtype=r):
        return nc.dram_tensor(name, shape, dtype, kind="ExternalInput").ap()

    def dout(name, shape, dtype):
        return nc.dram_tensor(name, shape, dtype, kind="ExternalOutput").ap()

    x_d = din("x_in", [26, TOK])                     # [f(+ones), t*b+j]
    encWhh_d = din("encWhh", [128, 4 * 2048])
    encWih_d = din("encWih", [26, 2048])
    decWhh_d = din("decWhh", [128, 4 * 2048])
    decWhy_d = din("decWhy", [128, 4 * 2048])
    decWy_d = din("decWy", [26, 2048])
    mlp1T_d = din("mlp1T", [128, 4 * 512])
    mlp2T_d = din("mlp2T", [128, 4 * 512])
    outWT_d = din("outWT", [128, 512])
    cr1T_d = din("cr1T", [26, 512])
    cr2T_d = din("cr2T", [128, 512])
    muT_d = din("muT", [128, 1024])
    lvT_d = din("lvT", [128, 1024])
    zdecT_d = din("zdecT", [128, 1024])
    ident_d = din("ident", [128, 128])
    mlp1b_d = din("mlp1b", [1, 512])
    mlp2b_d = din("mlp2b", [1, 512])
    outb_d = din("outb", [1, 128])
    ones_d = din("ones", [1, b2])
    mub_d = din("mub", [128, 2], f32)
    lvb_d = din("lvb", [128, 2], f32)
    zdecb_d = din("zdecb", [128, 4], f32)
    cr2b_d = din("cr2b", [128, 1], f32)
    cr1b_d = din("cr1b", [1, 512])

    pred_d = dout("pred", [25, TOK], r)
    c3d_d = dout("c3d", [C3D, TOK], f32)
    muo_d = dout("muo", [128, b2], f32)
    lvo_d = dout("lvo", [128, b2], f32)

    with tile.TileContext(nc) as tc:
        with tc.tile_pool(name="wp", bufs=1) as wp, \
             tc.tile_pool(name="bw", bufs=2) as bw, \
             tc.tile_pool(name="xp", bufs=3) as xp, \
             tc.tile_pool(name="yp", bufs=2) as yp, \
             tc.tile_pool(name="c3p", bufs=2) as c3p, \
             tc.tile_pool(name="ps", bufs=1, space="PSUM") as ps:

            PE, ACT, DVE, SYNC = nc.tensor, nc.scalar, nc.vector, nc.sync

            def wtile(name, shape, dtype=r):
                return wp.tile(shape, dtype, tag=name, name=name)

            # ---- persistent weights ----
            encWhh = bw.tile([128, 8192], r, tag="w")
            SYNC.dma_start(encWhh[:], encWhh_d[:])
            decWhh = bw.tile([128, 8192], r, tag="w")
            SYNC.dma_start(decWhh[:], decWhh_d[:])
            encWih = wtile("encWih", [26, 2048]); SYNC.dma_start(encWih[:], encWih_d[:])
            decWy = wtile("decWy", [26, 2048]); SYNC.dma_start(decWy[:], decWy_d[:])
            mlp1T = wtile("mlp1T", [128, 2048]); SYNC.dma_start(mlp1T[:], mlp1T_d[:])
            mlp2T = wtile("mlp2T", [128, 2048]); SYNC.dma_start(mlp2T[:], mlp2T_d[:])
            outWT = wtile("outWT", [128, 512]); SYNC.dma_start(outWT[:], outWT_d[:])
            cr1T = wtile("cr1T", [26, 512]); SYNC.dma_start(cr1T[:], cr1T_d[:])
            cr2T = wtile("cr2T", [128, 512]); SYNC.dma_start(cr2T[:], cr2T_d[:])
            muT = wtile("muT", [128, 1024]); SYNC.dma_start(muT[:], muT_d[:])
            lvT = wtile("lvT", [128, 1024]); SYNC.dma_start(lvT[:], lvT_d[:])
            zdecT = wtile("zdecT", [128, 1024]); SYNC.dma_start(zdecT[:], zdecT_d[:])
            ident = wtile("ident", [128, 128]); SYNC.dma_start(ident[:], ident_d[:])
            mlp1b = wtile("mlp1b", [1, 512]); SYNC.dma_start(mlp1b[:], mlp1b_d[:])
            mlp2b = wtile("mlp2b", [1, 512]); SYNC.dma_start(mlp2b[:], mlp2b_d[:])
            outb = wtile("outb", [1, 128]); SYNC.dma_start(outb[:], outb_d[:])
            ones = wtile("ones", [1, b2]); SYNC.dma_start(ones[:], ones_d[:])
            mub = wtile("mub", [128, 2], f32); SYNC.dma_start(mub[:], mub_d[:])
            lvb = wtile("lvb", [128, 2], f32); SYNC.dma_start(lvb[:], lvb_d[:])
            zdecb = wtile("zdecb", [128, 4], f32); SYNC.dma_start(zdecb[:], zdecb_d[:])
            cr2b = wtile("cr2b", [128, 1], f32); SYNC.dma_start(cr2b[:], cr2b_d[:])
            cr1b = wtile("cr1b", [1, 512]); SYNC.dma_start(cr1b[:], cr1b_d[:])

            # ---- persistent state ----
            # h ping-pong per half; c, sigmoid/tanh scratch per half
            hA = [wtile(f"hA{h}", [128, b2]) for h in (0, 1)]
            hB = [wtile(f"hB{h}", [128, b2]) for h in (0, 1)]
            cst = [wtile(f"c{h}", [128, b2], f32) for h in (0, 1)]
            sig = [[wtile(f"sig{q}{h}", [128, b2], f32) for h in (0, 1)]
                   for q in range(4)]
            m1 = [wtile(f"m1{h}", [128, b2]) for h in (0, 1)]
            m2 = [wtile(f"m2{h}", [128, b2]) for h in (0, 1)]
            hy = [wtile(f"hy{h}", [128, b2]) for h in (0, 1)]
            musb = wtile("musb", [128, b2])
            lvsb = wtile("lvsb", [128, b2], f32)
            baseT = [[wtile(f"base{q}{h}", [128, b2]) for h in (0, 1)]
                     for q in range(4)]
            yr = [wtile(f"yr{i}", [26, b]) for i in (0, 1)]
            zf = wtile("zf", [128, b2], f32)
            onef = wtile("onef", [26, b], f32)
            DVE.memset(zf[:], 0.0)
            DVE.memset(onef[:], 1.0)
            crm = [wtile(f"crm{m}", [128, b2]) for m in range(4)]

            GT = ps  # gate psum pool; tags g{q}{h}, one bank each

            def gate_psum(q, h):
                return GT.tile([128, b2], f32, tag=f"g{q}{h}", name=f"g{q}{h}")

            # init encoder state
            for h in (0, 1):
                DVE.tensor_copy(hA[h][:], zf[:])
                DVE.memset(cst[h][:], 0.0)

            # x stream prefetch
            xtiles = [None] * Wn

            def fetch_x(t):
                xt = xp.tile([26, b], r, tag="x", name="xt")
                SYNC.dma_start(xt[:], x_d[:, t * b:(t + 1) * b])
                xtiles[t] = xt

            fetch_x(0)
            fetch_x(1)

            QORD = (1, 0, 2, 3)  # f, i, g, o

            def lstm_gates(t, hp, hn, xt, whh, base_step):
                """One LSTM step's gates + elementwise, feature-major.

                base_step: None for encoder (x path via K=26 aug-matmul), else
                (decoder) adds baseT via identity matmul and y via decWy.
                """
                for h in (0, 1):
                    for q in QORD:
                        Tq = gate_psum(q, h)
                        if base_step is not None:
                            PE.matmul(Tq[:], lhsT=ident[:], rhs=baseT[q][h][:],
                                      start=True, stop=False)
                        for s in (0, 1):
                            am = q * 4 + 2 * h + s
                            dst = Tq[:, s * b:(s + 1) * b]
                            if base_step is None:
                                # h k=0 first (start), then x, then k=1..3
                                PE.matmul(dst, lhsT=whh[:, am * 128:(am + 1) * 128],
                                          rhs=hp[0][:, 0:b], start=True, stop=False)
                                PE.matmul(dst, lhsT=encWih[:, am * 128:(am + 1) * 128],
                                          rhs=xt[:], start=False, stop=False)
                                for k in (1, 2, 3):
                                    PE.matmul(
                                        dst,
                                        lhsT=whh[:, k * 2048 + am * 128:
                                                 k * 2048 + (am + 1) * 128],
                                        rhs=hp[k // 2][:, (k % 2) * b:(k % 2 + 1) * b],
                                        start=False, stop=(k == 3))
                            else:
                                PE.matmul(dst, lhsT=decWy[:, am * 128:(am + 1) * 128],
                                          rhs=xt[:], start=False, stop=False)
                                for k in (0, 1, 2, 3):
                                    PE.matmul(
                                        dst,
                                        lhsT=whh[:, k * 2048 + am * 128:
                                                 k * 2048 + (am + 1) * 128],
                                        rhs=hp[k // 2][:, (k % 2) * b:(k % 2 + 1) * b],
                                        start=False, stop=(k == 3))
                        ACT.activation(sig[q][h][:], Tq[:],
                                       AF.Tanh if q == 2 else AF.Sigmoid)
                        if q == 2:
                            # c = sig(f)*c + sig(i)*tanh(g); tanh(c) -> sig[2]
                            DVE.tensor_mul(sig[1][h][:], sig[1][h][:], cst[h][:])
                            DVE.tensor_mul(sig[0][h][:], sig[0][h][:], sig[2][h][:])
                            DVE.tensor_add(cst[h][:], sig[1][h][:], sig[0][h][:])
                            ACT.activation(sig[2][h][:], cst[h][:], AF.Tanh)
                    DVE.tensor_mul(hn[h][:], sig[3][h][:], sig[2][h][:])

            # ================= encoder =================
            for t in range(Wn):
                hp = hA if t % 2 == 0 else hB
                hn = hB if t % 2 == 0 else hA
                lstm_gates(t, hp, hn, xtiles[t], encWhh, None)
                if t + 2 < Wn:
                    fetch_x(t + 2)

            hx = hA if Wn % 2 == 0 else hB  # h after 50 steps

            # ================= latent =================
            Pm = GT.tile([128, b2], f32, tag="g00")
            for m in (0, 1):
                for k in range(4):
                    PE.matmul(Pm[:, m * b:(m + 1) * b],
                              lhsT=muT[:, (k * 2 + m) * 128:(k * 2 + m + 1) * 128],
                              rhs=hx[k // 2][:, (k % 2) * b:(k % 2 + 1) * b],
                              start=(k == 0), stop=(k == 3))
            for m in (0, 1):
                ACT.activation(musb[:, m * b:(m + 1) * b], Pm[:, m * b:(m + 1) * b],
                               AF.Identity, bias=mub[:, m:m + 1])
            SYNC.dma_start(muo_d[:], musb[:].bitcast(f32))

            Pl = GT.tile([128, b2], f32, tag="g10")
            for m in (0, 1):
                for k in range(4):
                    PE.matmul(Pl[:, m * b:(m + 1) * b],
                              lhsT=lvT[:, (k * 2 + m) * 128:(k * 2 + m + 1) * 128],
                              rhs=hx[k // 2][:, (k % 2) * b:(k % 2 + 1) * b],
                              start=(k == 0), stop=(k == 3))
            for m in (0, 1):
                ACT.activation(lvsb[:, m * b:(m + 1) * b], Pl[:, m * b:(m + 1) * b],
                               AF.Identity, bias=lvb[:, m:m + 1])
            SYNC.dma_start(lvo_d[:], lvsb[:])

            # h_y = zdec @ mu + zdec_b
            Pz = [GT.tile([128, b2], f32, tag=f"g2{h}", name=f"Pz{h}") for h in (0, 1)]
            for am in range(4):
                h, s = am // 2, am % 2
                for kz in (0, 1):
                    PE.matmul(Pz[h][:, s * b:(s + 1) * b],
                              lhsT=zdecT[:, (kz * 4 + am) * 128:(kz * 4 + am + 1) * 128],
                              rhs=musb[:, kz * b:(kz + 1) * b],
                              start=(kz == 0), stop=(kz == 1))
            for am in range(4):
                h, s = am // 2, am % 2
                ACT.activation(hy[h][:, s * b:(s + 1) * b], Pz[h][:, s * b:(s + 1) * b],
                               AF.Identity, bias=zdecb[:, am:am + 1])

            # ================= decoder base (Wih_hy @ h_y, const over t) =====
            decWhy = bw.tile([128, 8192], r, tag="w")
            SYNC.dma_start(decWhy[:], decWhy_d[:])
            for q in range(4):
                for h in (0, 1):
                    Tb = gate_psum(q, h)
                    for s in (0, 1):
                        am = q * 4 + 2 * h + s
                        for k in range(4):
                            PE.matmul(
                                Tb[:, s * b:(s + 1) * b],
                                lhsT=decWhy[:, k * 2048 + am * 128:
                                            k * 2048 + (am + 1) * 128],
                                rhs=hy[k // 2][:, (k % 2) * b:(k % 2 + 1) * b],
                                start=(k == 0), stop=(k == 3))
                    ACT.activation(baseT[q][h][:], Tb[:], AF.Copy)

            # ================= decoder =================
            for h in (0, 1):
                DVE.tensor_copy(hA[h][:], zf[:])
                DVE.memset(cst[h][:], 0.0)
            DVE.tensor_copy(yr[0][:], onef[:])
            DVE.tensor_copy(yr[0][0:25, :], zf[0:25, 0:b])
            DVE.tensor_copy(yr[1][:], onef[:])

            for t in range(Wn):
                hp = hA if t % 2 == 0 else hB
                hn = hB if t % 2 == 0 else hA
                yprev = yr[t % 2]
                ynext = yr[(t + 1) % 2]
                lstm_gates(t, hp, hn, yprev, decWhh, True)

                # mlp1 (borrow i-tags), mlp2 (borrow f-tags)
                for hh in (0, 1):
                    P1 = GT.tile([128, b2], f32, tag=f"g0{hh}", name="P1")
                    for s in (0, 1):
                        mm = 2 * hh + s
                        dst = P1[:, s * b:(s + 1) * b]
                        PE.matmul(dst, lhsT=mlp1b[0:1, mm * 128:(mm + 1) * 128],
                                  rhs=ones[0:1, 0:b], start=True, stop=False)
                        for k in range(4):
                            PE.matmul(dst,
                                      lhsT=mlp1T[:, k * 512 + mm * 128:
                                                 k * 512 + (mm + 1) * 128],
                                      rhs=hn[k // 2][:, (k % 2) * b:(k % 2 + 1) * b],
                                      start=False, stop=(k == 3))
                    ACT.activation(m1[hh][:], P1[:], AF.Tanh)
                for hh in (0, 1):
                    P2 = GT.tile([128, b2], f32, tag=f"g1{hh}", name="P2")
                    for s in (0, 1):
                        mm = 2 * hh + s
                        dst = P2[:, s * b:(s + 1) * b]
                        PE.matmul(dst, lhsT=mlp2b[0:1, mm * 128:(mm + 1) * 128],
                                  rhs=ones[0:1, 0:b], start=True, stop=False)
                        for k in range(4):
                            PE.matmul(dst,
                                      lhsT=mlp2T[:, k * 512 + mm * 128:
                                                 k * 512 + (mm + 1) * 128],
                                      rhs=m1[k // 2][:, (k % 2) * b:(k % 2 + 1) * b],
                                      start=False, stop=(k == 3))
                    ACT.activation(m2[hh][:], P2[:], AF.Tanh)

                Po = GT.tile([128, b], f32, tag="g20")
                PE.matmul(Po[:], lhsT=outb[0:1, :], rhs=ones[0:1, 0:b],
                          start=True, stop=False)
                for k in range(4):
                    PE.matmul(Po[:], lhsT=outWT[:, k * 128:(k + 1) * 128],
                              rhs=m2[k // 2][:, (k % 2) * b:(k % 2 + 1) * b],
                              start=False, stop=(k == 3))
                DVE.tensor_copy(ynext[0:25, :], Po[0:25, :])
                SYNC.dma_start(pred_d[:, t * b:(t + 1) * b], ynext[0:25, :])

            # ================= 3dmm head =================
            NT = TOK // b2  # 25 tiles of 480 tokens
            for i in range(NT):
                ys = yp.tile([25, b2], r, tag="ys", name="ys")
                SYNC.dma_start(ys[:], pred_d[:, i * b2:(i + 1) * b2])
                for mm in range(4):
                    tag = ("g00", "g01", "g10", "g11")[mm]
                    Pc = GT.tile([128, b2], f32, tag=tag)
                    PE.matmul(Pc[:], lhsT=cr1b[0:1, mm * 128:(mm + 1) * 128],
                              rhs=ones[0:1, :], start=True, stop=False)
                    PE.matmul(Pc[:], lhsT=cr1T[0:25, mm * 128:(mm + 1) * 128],
                              rhs=ys[:], start=False, stop=True)
                    ACT.activation(crm[mm][:], Pc[:], AF.Tanh)
                P2 = GT.tile([128, b2], f32, tag="g20", name="P2c")
                for k in range(4):
                    PE.matmul(P2[:], lhsT=cr2T[:, k * 128:(k + 1) * 128],
                              rhs=crm[k][:], start=(k == 0), stop=(k == 3))
                c3t = c3p.tile([C3D, b2], f32, tag="c3", name="c3t")
                DVE.tensor_scalar_add(c3t[:], P2[0:C3D, :], cr2b[0:C3D, 0:1])
                SYNC.dma_start(c3d_d[:, i * b2:(i + 1) * b2], c3t[:])

    n_split = _split_multiwait(nc)
    # sanity: nothing may carry >1 wait
    for f in nc.m.functions:
        for bb in f.blocks:
            for inst in bb.instructions:
                si = getattr(inst, "sync_info", None)
                if si and si.on_wait and len(si.on_wait) > 1:
                    raise RuntimeError(
                        f"instruction {inst.name} {type(inst).__name__} has "
                        f"{len(si.on_wait)} waits")
    return nc


def _pack_weights(inp):
    f = np.float32

    def chunkT(wT, nk, nm):
        # wT: [K, M] -> [128, nk*nm*128] laid out [p, k*Mfull + m*128 + mcol]
        K, Mm = wT.shape
        assert K == nk * 128 and Mm == nm * 128
        return np.ascontiguousarray(
            wT.reshape(nk, 128, Mm).transpose(1, 0, 2).reshape(128, nk * Mm)
        ).astype(f)

    encWhh = chunkT(np.asarray(inp["enc_Whh"], f).T, 4, 16)
    decWhh = chunkT(np.asarray(inp["dec_Whh"], f).T, 4, 16)
    decWhy = chunkT(np.asarray(inp["dec_Wih"], f)[:, :H].T, 4, 16)
    mlp1T = chunkT(np.asarray(inp["mlp1_W"], f).T, 4, 4)
    mlp2T = chunkT(np.asarray(inp["mlp2_W"], f).T, 4, 4)

    encWih = np.zeros((26, 2048), f)
    encWih[:25] = np.asarray(inp["enc_Wih"], f).T
    encWih[25] = np.asarray(inp["enc_bih"], f) + np.asarray(inp["enc_bhh"], f)
    decWy = np.zeros((26, 2048), f)
    decWy[:25] = np.asarray(inp["dec_Wih"], f)[:, H:].T
    decWy[25] = np.asarray(inp["dec_bih"], f) + np.asarray(inp["dec_bhh"], f)

    outWT_full = np.zeros((M, 128), f)
    outWT_full[:, :F] = np.asarray(inp["out_W"], f).T
    outWT = chunkT(outWT_full, 4, 1)

    cr1T = np.zeros((26, 512), f)
    cr1T[:25] = np.asarray(inp["cr1_W"], f).T
    cr1T[25] = np.asarray(inp["cr1_b"], f)

    cr2T_full = np.zeros((H, 128), f)
    cr2T_full[:, :C3D] = np.asarray(inp["cr2_W"], f).T
    cr2T = chunkT(cr2T_full, 4, 1)

    muT = chunkT(np.asarray(inp["mu_W"], f).T, 4, 2)
    lvT = chunkT(np.asarray(inp["lv_W"], f).T, 4, 2)
    zdecT = chunkT(np.asarray(inp["zdec_W"], f).T, 2, 4)

    outb = np.zeros((1, 128), f)
    outb[0, :F] = np.asarray(inp["out_b"], f)
    cr2b = np.zeros((128, 1), f)
    cr2b[:C3D, 0] = np.asarray(inp["cr2_b"], f)

    return {
        "encWhh": encWhh, "encWih": encWih,
        "decWhh": decWhh, "decWhy": decWhy, "decWy": decWy,
        "mlp1T": mlp1T, "mlp2T": mlp2T, "outWT": outWT,
        "cr1T": cr1T, "cr2T": cr2T,
        "muT": muT, "lvT": lvT, "zdecT": zdecT,
        "ident": np.eye(128, dtype=f),
        "mlp1b": np.asarray(inp["mlp1_b"], f)[None, :],
        "mlp2b": np.asarray(inp["mlp2_b"], f)[None, :],
        "outb": outb,
        "ones": np.ones((1, b2), f),
        "mub": np.ascontiguousarray(np.asarray(inp["mu_b"], f).reshape(2, 128).T),
        "lvb": np.ascontiguousarray(np.asarray(inp["lv_b"], f).reshape(2, 128).T),
        "zdecb": np.ascontiguousarray(np.asarray(inp["zdec_b"], f).reshape(4, 128).T),
        "cr2b": cr2b,
        "cr1b": np.asarray(inp["cr1_b"], np.float32)[None, :],
    }


def kernel(**inputs):
    from concourse.bass_utils import run_bass_kernel_spmd

    trace = os.environ.get("BASS_LSTM_TRACE") == "1"
    if trace:
        _install_ntff_hook()

    if "nc" not in _cache:
        _cache["nc"] = _build_program()
    nc = _cache["nc"]

    wmap = _pack_weights(inputs)
    le = np.asarray(inputs["listener_emotion"], np.float32)
    wnd = le.reshape(B * (T // Wn), Wn, F)  # (1920, 50, 25)

    in_maps = []
    for k in range(NCORE):
        xw = wnd[b * k: b * (k + 1)]        # (240, 50, 25)
        xT = np.empty((26, Wn, b), np.float32)
        xT[:25] = xw.transpose(2, 1, 0)
        xT[25] = 1.0
        in_maps.append({"x_in": np.ascontiguousarray(xT.reshape(26, TOK)), **wmap})

    res = run_bass_kernel_spmd(nc, in_maps, list(range(NCORE)), trace=trace)
    if trace:
        kernel._last_exec_ns = res.exec_time_ns
        kernel._last_mean_exec_ns = res.mean_exec_time_ns

    preds, c3ds, mus, lvs = [], [], [], []
    for k in range(NCORE):
        o = res.results[k]
        pk = o["pred"].reshape(25, Wn, b).transpose(2, 1, 0)   # (j, t, f)
        preds.append(pk.reshape(b // 15, 15 * Wn, F))
        ck = o["c3d"].reshape(C3D, Wn, b).transpose(2, 1, 0)
        c3ds.append(ck.reshape(b // 15, 15 * Wn, C3D))
        mus.append(o["muo"].reshape(128, 2, b).transpose(2, 1, 0).reshape(b, Z))
        lvs.append(o["lvo"].reshape(128, 2, b).transpose(2, 1, 0).reshape(b, Z))

    prediction = np.concatenate(preds, 0).astype(np.float32)
    coeff_3dmm = np.concatenate(c3ds, 0).astype(np.float32)
    mu = np.concatenate(mus, 0).astype(np.float32)
    logvar = np.concatenate(lvs, 0).astype(np.float32)
    return prediction, coeff_3dmm, mu, logvar


# revision 9
# speedup vs baseline: 2.0695x; 2.0695x over previous
"""Trainium2 Bass kernel for nn_AutoencoderRNN_VAE_v2 (LSTM VAE autoencoder).

Data-parallel over the 1920 effective windows: 240 windows per NeuronCore.
All activations live feature-major ([feature_chunk<=128, batch] "transposed"
layout) so the LSTM recurrence needs no transposes. Matmuls run in float16
(1 cyc/row, hideable weight loads, ~1e-3 end-to-end error) with fp32 PSUM
accumulation.

Self-contained: hardcodes all shapes; builds + compiles the SPMD program on
first call and runs it on cores 0-7.
"""
import os
import sys
import types

sys.path.insert(0, "/opt/trn_rl_repo")
import numpy as np

B, T, Wn, F, H, Z, M, C3D = 128, 750, 50, 25, 512, 256, 512, 58
NCORE = 8
b = 240            # windows per core
b2 = 2 * b         # 480, merged half-tile free width
TOK = Wn * b       # 12000 tokens per core

_cache = {}


def _install_ntff_hook():
    try:
        import antenv
        from trn_agent_boot.trn_boot import _ntff_profile_via_ctypes
        hook = _ntff_profile_via_ctypes("/opt/axon/libaxon_pjrt.so")
        mod = types.ModuleType("antenv.axon_hooks")
        mod.get_axon_ntff_profile_hook = lambda: hook
        mod.set_axon_ntff_profile_hook = lambda h: None
        sys.modules["antenv.axon_hooks"] = mod
        antenv.axon_hooks = mod
    except Exception:
        pass


def _patch_tile_drain():
    """Stock kernel-tail drain carries one wait per live semaphore (11+),
    overflowing walrus's 1-wait-per-instruction budget. Split it."""
    import concourse.tile as tile
    from concourse.vector_clock import ScopedClock, VectorClock

    def _split_drain_and_barrier(self, tick_clock, wait_clock):
        vec = list(tick_clock.global_clock)
        for i, t in enumerate(vec):
            if t <= 0:
                continue
            mv = [0] * len(vec)
            mv[i] = t
            d = self.nc.sync.drain()
            wait_clock.add_sem_waits(d.ins, ScopedClock({None: VectorClock(mv)}))
        self.nc.all_engine_barrier()
        assert self.sems is not None
        popped = self.nc._tile_sem_poison_stack.pop()
        assert popped is self._sem_poison
        self.nc.clear_and_free_semaphores(list(self.sems.allocated().values()))
        self.nc.all_engine_barrier()

    tile.TileContext._drain_and_barrier = _split_drain_and_barrier


def _split_multiwait(nc):
    """walrus (this toolchain) accepts only ONE sync wait per TPB instruction.
    Hoist extra waits onto injected same-engine NOPs just before the
    offending instruction."""
    from concourse.mybir import SyncInfo

    def make_nop(engine):
        ins = nc.engines[engine].nop(nofuse=True).ins
        cur = nc.cur_bb.bb.instructions
        assert cur[-1] is ins
        cur.pop()
        return ins

    n_split = 0
    for f in nc.m.functions:
        for bb in f.blocks:
            new = []
            for inst in bb.instructions:
                si = getattr(inst, "sync_info", None)
                waits = list(si.on_wait) if si and si.on_wait else []
                if len(waits) > 1 and getattr(inst, "engine", None) is not None:
                    for w in waits[:-1]:
                        nop = make_nop(inst.engine)
                        nop.sync_info = SyncInfo(on_wait=[w], on_update=[])
                        new.append(nop)
                    inst.sync_info = SyncInfo(
                        on_wait=[waits[-1]], on_update=list(si.on_update or [])
                    )
                    n_split += 1
                new.append(inst)
            bb.instructions[:] = new
    return n_split


def _build_program():
    import concourse.bass as bass
    import concourse.mybir as mybir
    import concourse.tile as tile

    _patch_tile_drain()

    dt = mybir.dt
    r = dt.float16
    f32 = dt.float32
    AF = mybir.ActivationFunctionType

    nc = bass.Bass(trn_type="TRN2", target_bir_lowering=False, debug=False)

    def din(name, shape, dtype=r):
        return nc.dram_tensor(name, shape, dtype, kind="ExternalInput").ap()

    def dout(name, shape, dtype):
        return nc.dram_tensor(name, shape, dtype, kind="ExternalOutput").ap()

    x_d = din("x_in", [26, TOK])                     # [f(+ones), t*b+j]
    encWhh_d = din("encWhh", [128, 4 * 2048])
    encWih_d = din("encWih", [26, 2048])
    decWhh_d = din("decWhh", [128, 4 * 2048])
    decWhy_d = din("decWhy", [128, 4 * 2048])
    decWy_d = din("decWy", [26, 2048])
    mlp1T_d = din("mlp1T", [128, 4 * 512])
    mlp2T_d = din("mlp2T", [128, 4 * 512])
    outWT_d = din("outWT", [128, 512])
    cr1T_d = din("cr1T", [26, 512])
    cr2T_d = din("cr2T", [128, 512])
    muT_d = din("muT", [128, 1024])
    lvT_d = din("lvT", [128, 1024])
    zdecT_d = din("zdecT", [128, 1024])
    ident_d = din("ident", [128, 128])
    mlp1b_d = din("mlp1b", [1, 512])
    mlp2b_d = din("mlp2b", [1, 512])
    outb_d = din("outb", [1, 128])
    ones_d = din("ones", [1, b2])
    mub_d = din("mub", [128, 2], f32)
    lvb_d = din("lvb", [128, 2], f32)
    zdecb_d = din("zdecb", [128, 4], f32)
    cr2b_d = din("cr2b", [128, 1], f32)
    cr1b_d = din("cr1b", [1, 512])

    pred_d = dout("pred", [25, TOK], r)
    c3d_d = dout("c3d", [C3D, TOK], f32)
    muo_d = dout("muo", [128, b2], f32)
    lvo_d = dout("lvo", [128, b2], f32)

    with tile.TileContext(nc) as tc:
        with tc.tile_pool(name="wp", bufs=1) as wp, \
             tc.tile_pool(name="bw", bufs=2) as bw, \
             tc.tile_pool(name="xp", bufs=3) as xp, \
             tc.tile_pool(name="yp", bufs=2) as yp, \
             tc.tile_pool(name="c3p", bufs=2) as c3p, \
             tc.tile_pool(name="ps", bufs=1, space="PSUM") as ps:

            PE, ACT, DVE, SYNC = nc.tensor, nc.scalar, nc.vector, nc.sync

            def wtile(name, shape, dtype=r):
                return wp.tile(shape, dtype, tag=name, name=name)

            # ---- persistent weights ----
            encWhh = bw.tile([128, 8192], r, tag="w")
            SYNC.dma_start(encWhh[:], encWhh_d[:])
            decWhh = bw.tile([128, 8192], r, tag="w")
            SYNC.dma_start(decWhh[:], decWhh_d[:])
            encWih = wtile("encWih", [26, 2048]); SYNC.dma_start(encWih[:], encWih_d[:])
            decWy = wtile("decWy", [26, 2048]); SYNC.dma_start(decWy[:], decWy_d[:])
            mlp1T = wtile("mlp1T", [128, 2048]); SYNC.dma_start(mlp1T[:], mlp1T_d[:])
            mlp2T = wtile("mlp2T", [128, 2048]); SYNC.dma_start(mlp2T[:], mlp2T_d[:])
            outWT = wtile("outWT", [128, 512]); SYNC.dma_start(outWT[:], outWT_d[:])
            cr1T = wtile("cr1T", [26, 512]); SYNC.dma_start(cr1T[:], cr1T_d[:])
            cr2T = wtile("cr2T", [128, 512]); SYNC.dma_start(cr2T[:], cr2T_d[:])
            muT = wtile("muT", [128, 1024]); SYNC.dma_start(muT[:], muT_d[:])
            lvT = wtile("lvT", [128, 1024]); SYNC.dma_start(lvT[:], lvT_d[:])
            zdecT = wtile("zdecT", [128, 1024]); SYNC.dma_start(zdecT[:], zdecT_d[:])
            ident = wtile("ident", [128, 128]); SYNC.dma_start(ident[:], ident_d[:])
            mlp1b = wtile("mlp1b", [1, 512]); SYNC.dma_start(mlp1b[:], mlp1b_d[:])
            mlp2b = wtile("mlp2b", [1, 512]); SYNC.dma_start(mlp2b[:], mlp2b_d[:])
            outb = wtile("outb", [1, 128]); SYNC.dma_start(outb[:], outb_d[:])
            ones = wtile("ones", [1, b2]); SYNC.dma_start(ones[:], ones_d[:])
            mub = wtile("mub", [128, 2], f32); SYNC.dma_start(mub[:], mub_d[:])
            lvb = wtile("lvb", [128, 2], f32); SYNC.dma_start(lvb[:], lvb_d[:])
            zdecb = wtile("zdecb", [128, 4], f32); SYNC.dma_start(zdecb[:], zdecb_d[:])
            cr2b = wtile("cr2b", [128, 1], f32); SYNC.dma_start(cr2b[:], cr2b_d[:])
            cr1b = wtile("cr1b", [1, 512]); SYNC.dma_start(cr1b[:], cr1b_d[:])

            # ---- persistent state ----
            # h ping-pong per half; c, sigmoid/tanh scratch per half
            hA = [wtile(f"hA{h}", [128, b2]) for h in (0, 1)]
            hB = [wtile(f"hB{h}", [128, b2]) for h in (0, 1)]
            cst = [wtile(f"c{h}", [128, b2], f32) for h in (0, 1)]
            sig = [[wtile(f"sig{q}{h}", [128, b2], f32) for h in (0, 1)]
                   for q in range(4)]
            m1 = [wtile(f"m1{h}", [128, b2]) for h in (0, 1)]
            m2 = [wtile(f"m2{h}", [128, b2]) for h in (0, 1)]
            hy = [wtile(f"hy{h}", [128, b2]) for h in (0, 1)]
            musb = wtile("musb", [128, b2])
            musb32 = wtile("musb32", [128, b2], f32)
            lvsb = wtile("lvsb", [128, b2], f32)
            baseT = [[wtile(f"base{q}{h}", [128, b2]) for h in (0, 1)]
                     for q in range(4)]
            yr = [wtile(f"yr{i}", [26, b]) for i in (0, 1)]
            zf = wtile("zf", [128, b2], f32)
            onef = wtile("onef", [26, b], f32)
            DVE.memset(zf[:], 0.0)
            DVE.memset(onef[:], 1.0)
            crm = [wtile(f"crm{m}", [128, b2]) for m in range(4)]

            GT = ps  # gate psum pool; tags g{q}{h}, one bank each

            def gate_psum(q, h):
                return GT.tile([128, b2], f32, tag=f"g{q}{h}", name=f"g{q}{h}")

            # init encoder state
            for h in (0, 1):
                DVE.tensor_copy(hA[h][:], zf[:])
                DVE.memset(cst[h][:], 0.0)

            # x stream prefetch
            xtiles = [None] * Wn

            def fetch_x(t):
                xt = xp.tile([26, b], r, tag="x", name="xt")
                SYNC.dma_start(xt[:], x_d[:, t * b:(t + 1) * b])
                xtiles[t] = xt

            fetch_x(0)
            fetch_x(1)

            QORD = (1, 0, 2, 3)  # f, i, g, o

            def lstm_gates(t, hp, hn, xt, whh, base_step):
                """One LSTM step's gates + elementwise, feature-major.

                base_step: None for encoder (x path via K=26 aug-matmul), else
                (decoder) adds baseT via identity matmul and y via decWy.
                """
                for h in (0, 1):
                    for q in QORD:
                        Tq = gate_psum(q, h)
                        if base_step is not None:
                            PE.matmul(Tq[:], lhsT=ident[:], rhs=baseT[q][h][:],
                                      start=True, stop=False)
                        for s in (0, 1):
                            am = q * 4 + 2 * h + s
                            dst = Tq[:, s * b:(s + 1) * b]
                            if base_step is None:
                                # h k=0 first (start), then x, then k=1..3
                                PE.matmul(dst, lhsT=whh[:, am * 128:(am + 1) * 128],
                                          rhs=hp[0][:, 0:b], start=True, stop=False)
                                PE.matmul(dst, lhsT=encWih[:, am * 128:(am + 1) * 128],
                                          rhs=xt[:], start=False, stop=False)
                                for k in (1, 2, 3):
                                    PE.matmul(
                                        dst,
                                        lhsT=whh[:, k * 2048 + am * 128:
                                                 k * 2048 + (am + 1) * 128],
                                        rhs=hp[k // 2][:, (k % 2) * b:(k % 2 + 1) * b],
                                        start=False, stop=(k == 3))
                            else:
                                PE.matmul(dst, lhsT=decWy[:, am * 128:(am + 1) * 128],
                                          rhs=xt[:], start=False, stop=False)
                                for k in (0, 1, 2, 3):
                                    PE.matmul(
                                        dst,
                                        lhsT=whh[:, k * 2048 + am * 128:
                                                 k * 2048 + (am + 1) * 128],
                                        rhs=hp[k // 2][:, (k % 2) * b:(k % 2 + 1) * b],
                                        start=False, stop=(k == 3))
                        ACT.activation(sig[q][h][:], Tq[:],
                                       AF.Tanh if q == 2 else AF.Sigmoid)
                        if q == 2:
                            # c = sig(f)*c + sig(i)*tanh(g); tanh(c) -> sig[2]
                            DVE.tensor_mul(sig[1][h][:], sig[1][h][:], cst[h][:])
                            DVE.tensor_mul(sig[0][h][:], sig[0][h][:], sig[2][h][:])
                            DVE.tensor_add(cst[h][:], sig[1][h][:], sig[0][h][:])
                            ACT.activation(sig[2][h][:], cst[h][:], AF.Tanh)
                    DVE.tensor_mul(hn[h][:], sig[3][h][:], sig[2][h][:])

            # ================= encoder =================
            for t in range(Wn):
                hp = hA if t % 2 == 0 else hB
                hn = hB if t % 2 == 0 else hA
                lstm_gates(t, hp, hn, xtiles[t], encWhh, None)
                if t + 2 < Wn:
                    fetch_x(t + 2)

            hx = hA if Wn % 2 == 0 else hB  # h after 50 steps

            # ================= latent =================
            Pm = GT.tile([128, b2], f32, tag="g00")
            for m in (0, 1):
                for k in range(4):
                    PE.matmul(Pm[:, m * b:(m + 1) * b],
                              lhsT=muT[:, (k * 2 + m) * 128:(k * 2 + m + 1) * 128],
                              rhs=hx[k // 2][:, (k % 2) * b:(k % 2 + 1) * b],
                              start=(k == 0), stop=(k == 3))
            for m in (0, 1):
                ACT.activation(musb32[:, m * b:(m + 1) * b], Pm[:, m * b:(m + 1) * b],
                               AF.Identity, bias=mub[:, m:m + 1])
            DVE.tensor_copy(musb[:], musb32[:])
            SYNC.dma_start(muo_d[:], musb32[:])

            Pl = GT.tile([128, b2], f32, tag="g10")
            for m in (0, 1):
                for k in range(4):
                    PE.matmul(Pl[:, m * b:(m + 1) * b],
                              lhsT=lvT[:, (k * 2 + m) * 128:(k * 2 + m + 1) * 128],
                              rhs=hx[k // 2][:, (k % 2) * b:(k % 2 + 1) * b],
                              start=(k == 0), stop=(k == 3))
            for m in (0, 1):
                ACT.activation(lvsb[:, m * b:(m + 1) * b], Pl[:, m * b:(m + 1) * b],
                               AF.Identity, bias=lvb[:, m:m + 1])
            SYNC.dma_start(lvo_d[:], lvsb[:])

            # h_y = zdec @ mu + zdec_b
            Pz = [GT.tile([128, b2], f32, tag=f"g2{h}", name=f"Pz{h}") for h in (0, 1)]
            for am in range(4):
                h, s = am // 2, am % 2
                for kz in (0, 1):
                    PE.matmul(Pz[h][:, s * b:(s + 1) * b],
                              lhsT=zdecT[:, (kz * 4 + am) * 128:(kz * 4 + am + 1) * 128],
                              rhs=musb[:, kz * b:(kz + 1) * b],
                              start=(kz == 0), stop=(kz == 1))
            for am in range(4):
                h, s = am // 2, am % 2
                ACT.activation(hy[h][:, s * b:(s + 1) * b], Pz[h][:, s * b:(s + 1) * b],
                               AF.Identity, bias=zdecb[:, am:am + 1])

            # ================= decoder base (Wih_hy @ h_y, const over t) =====
            decWhy = bw.tile([128, 8192], r, tag="w")
            SYNC.dma_start(decWhy[:], decWhy_d[:])
            for q in range(4):
                for h in (0, 1):
                    Tb = gate_psum(q, h)
                    for s in (0, 1):
                        am = q * 4 + 2 * h + s
                        for k in range(4):
                            PE.matmul(
                                Tb[:, s * b:(s + 1) * b],
                                lhsT=decWhy[:, k * 2048 + am * 128:
                                            k * 2048 + (am + 1) * 128],
                                rhs=hy[k // 2][:, (k % 2) * b:(k % 2 + 1) * b],
                                start=(k == 0), stop=(k == 3))
                    ACT.activation(baseT[q][h][:], Tb[:], AF.Copy)

            # ================= decoder =================
            for h in (0, 1):
                DVE.tensor_copy(hA[h][:], zf[:])
                DVE.memset(cst[h][:], 0.0)
            DVE.tensor_copy(yr[0][:], onef[:])
            DVE.tensor_copy(yr[0][0:25, :], zf[0:25, 0:b])
            DVE.tensor_copy(yr[1][:], onef[:])

            for t in range(Wn):
                hp = hA if t % 2 == 0 else hB
                hn = hB if t % 2 == 0 else hA
                yprev = yr[t % 2]
                ynext = yr[(t + 1) % 2]
                lstm_gates(t, hp, hn, yprev, decWhh, True)

                # mlp1 (borrow i-tags), mlp2 (borrow f-tags)
                for hh in (0, 1):
                    P1 = GT.tile([128, b2], f32, tag=f"g0{hh}", name="P1")
                    for s in (0, 1):
                        mm = 2 * hh + s
                        dst = P1[:, s * b:(s + 1) * b]
                        PE.matmul(dst, lhsT=mlp1b[0:1, mm * 128:(mm + 1) * 128],
                                  rhs=ones[0:1, 0:b], start=True, stop=False)
                        for k in range(4):
                            PE.matmul(dst,
                                      lhsT=mlp1T[:, k * 512 + mm * 128:
                                                 k * 512 + (mm + 1) * 128],
                                      rhs=hn[k // 2][:, (k % 2) * b:(k % 2 + 1) * b],
                                      start=False, stop=(k == 3))
                    ACT.activation(m1[hh][:], P1[:], AF.Tanh)
                for hh in (0, 1):
                    P2 = GT.tile([128, b2], f32, tag=f"g1{hh}", name="P2")
                    for s in (0, 1):
                        mm = 2 * hh + s
                        dst = P2[:, s * b:(s + 1) * b]
                        PE.matmul(dst, lhsT=mlp2b[0:1, mm * 128:(mm + 1) * 128],
                                  rhs=ones[0:1, 0:b], start=True, stop=False)
                        for k in range(4):
                            PE.matmul(dst,
                                      lhsT=mlp2T[:, k * 512 + mm * 128:
                                                 k * 512 + (mm + 1) * 128],
                                      rhs=m1[k // 2][:, (k % 2) * b:(k % 2 + 1) * b],
                                      start=False, stop=(k == 3))
                    ACT.activation(m2[hh][:], P2[:], AF.Tanh)

                Po = GT.tile([128, b], f32, tag="g20")
                PE.matmul(Po[:], lhsT=outb[0:1, :], rhs=ones[0:1, 0:b],
                          start=True, stop=False)
                for k in range(4):
                    PE.matmul(Po[:], lhsT=outWT[:, k * 128:(k + 1) * 128],
                              rhs=m2[k // 2][:, (k % 2) * b:(k % 2 + 1) * b],
                              start=False, stop=(k == 3))
                DVE.tensor_copy(ynext[0:25, :], Po[0:25, :])
                SYNC.dma_start(pred_d[:, t * b:(t + 1) * b], ynext[0:25, :])

            # ================= 3dmm head =================
            NT = TOK // b2  # 25 tiles of 480 tokens
            for i in range(NT):
                ys = yp.tile([25, b2], r, tag="ys", name="ys")
                SYNC.dma_start(ys[:], pred_d[:, i * b2:(i + 1) * b2])
                for mm in range(4):
                    tag = ("g00", "g01", "g10", "g11")[mm]
                    Pc = GT.tile([128, b2], f32, tag=tag)
                    PE.matmul(Pc[:], lhsT=cr1b[0:1, mm * 128:(mm + 1) * 128],
                              rhs=ones[0:1, :], start=True, stop=False)
                    PE.matmul(Pc[:], lhsT=cr1T[0:25, mm * 128:(mm + 1) * 128],
                              rhs=ys[:], start=False, stop=True)
                    ACT.activation(crm[mm][:], Pc[:], AF.Tanh)
                P2 = GT.tile([128, b2], f32, tag="g20", name="P2c")
                for k in range(4):
                    PE.matmul(P2[:], lhsT=cr2T[:, k * 128:(k + 1) * 128],
                              rhs=crm[k][:], start=(k == 0), stop=(k == 3))
                c3t = c3p.tile([C3D, b2], f32, tag="c3", name="c3t")
                DVE.tensor_scalar_add(c3t[:], P2[0:C3D, :], cr2b[0:C3D, 0:1])
                SYNC.dma_start(c3d_d[:, i * b2:(i + 1) * b2], c3t[:])

    n_split = _split_multiwait(nc)
    # sanity: nothing may carry >1 wait
    for f in nc.m.functions:
        for bb in f.blocks:
            for inst in bb.instructions:
                si = getattr(inst, "sync_info", None)
                if si and si.on_wait and len(si.on_wait) > 1:
                    raise RuntimeError(
                        f"instruction {inst.name} {type(inst).__name__} has "
                        f"{len(si.on_wait)} waits")
    return nc


def _pack_weights(inp):
    f = np.float32
    h16 = np.float16

    def chunkT(wT, nk, nm):
        # wT: [K, M] -> [128, nk*nm*128] laid out [p, k*Mfull + m*128 + mcol]
        K, Mm = wT.shape
        assert K == nk * 128 and Mm == nm * 128
        return np.ascontiguousarray(
            wT.reshape(nk, 128, Mm).transpose(1, 0, 2).reshape(128, nk * Mm)
        ).astype(f)

    encWhh = chunkT(np.asarray(inp["enc_Whh"], f).T, 4, 16)
    decWhh = chunkT(np.asarray(inp["dec_Whh"], f).T, 4, 16)
    decWhy = chunkT(np.asarray(inp["dec_Wih"], f)[:, :H].T, 4, 16)
    mlp1T = chunkT(np.asarray(inp["mlp1_W"], f).T, 4, 4)
    mlp2T = chunkT(np.asarray(inp["mlp2_W"], f).T, 4, 4)

    encWih = np.zeros((26, 2048), f)
    encWih[:25] = np.asarray(inp["enc_Wih"], f).T
    encWih[25] = np.asarray(inp["enc_bih"], f) + np.asarray(inp["enc_bhh"], f)
    decWy = np.zeros((26, 2048), f)
    decWy[:25] = np.asarray(inp["dec_Wih"], f)[:, H:].T
    decWy[25] = np.asarray(inp["dec_bih"], f) + np.asarray(inp["dec_bhh"], f)

    outWT_full = np.zeros((M, 128), f)
    outWT_full[:, :F] = np.asarray(inp["out_W"], f).T
    outWT = chunkT(outWT_full, 4, 1)

    cr1T = np.zeros((26, 512), f)
    cr1T[:25] = np.asarray(inp["cr1_W"], f).T
    cr1T[25] = np.asarray(inp["cr1_b"], f)

    cr2T_full = np.zeros((H, 128), f)
    cr2T_full[:, :C3D] = np.asarray(inp["cr2_W"], f).T
    cr2T = chunkT(cr2T_full, 4, 1)

    muT = chunkT(np.asarray(inp["mu_W"], f).T, 4, 2)
    lvT = chunkT(np.asarray(inp["lv_W"], f).T, 4, 2)
    zdecT = chunkT(np.asarray(inp["zdec_W"], f).T, 2, 4)

    outb = np.zeros((1, 128), f)
    outb[0, :F] = np.asarray(inp["out_b"], f)
    cr2b = np.zeros((128, 1), f)
    cr2b[:C3D, 0] = np.asarray(inp["cr2_b"], f)

    return {
        "encWhh": encWhh.astype(h16), "encWih": encWih.astype(h16),
        "decWhh": decWhh.astype(h16), "decWhy": decWhy.astype(h16),
        "decWy": decWy.astype(h16),
        "mlp1T": mlp1T.astype(h16), "mlp2T": mlp2T.astype(h16),
        "outWT": outWT.astype(h16),
        "cr1T": cr1T.astype(h16), "cr2T": cr2T.astype(h16),
        "muT": muT.astype(h16), "lvT": lvT.astype(h16),
        "zdecT": zdecT.astype(h16),
        "ident": np.eye(128, dtype=h16),
        "mlp1b": np.asarray(inp["mlp1_b"], f)[None, :].astype(h16),
        "mlp2b": np.asarray(inp["mlp2_b"], f)[None, :].astype(h16),
        "outb": outb.astype(h16),
        "ones": np.ones((1, b2), h16),
        "mub": np.ascontiguousarray(np.asarray(inp["mu_b"], f).reshape(2, 128).T),
        "lvb": np.ascontiguousarray(np.asarray(inp["lv_b"], f).reshape(2, 128).T),
        "zdecb": np.ascontiguousarray(np.asarray(inp["zdec_b"], f).reshape(4, 128).T),
        "cr2b": cr2b,
        "cr1b": np.asarray(inp["cr1_b"], np.float32)[None, :].astype(h16),
    }


def kernel(**inputs):
    from concourse.bass_utils import run_bass_kernel_spmd

    trace = os.environ.get("BASS_LSTM_TRACE") == "1"
    if trace:
        _install_ntff_hook()

    if "nc" not in _cache:
        _cache["nc"] = _build_program()
    nc = _cache["nc"]

    wmap = _pack_weights(inputs)
    le = np.asarray(inputs["listener_emotion"], np.float32)
    wnd = le.reshape(B * (T // Wn), Wn, F)  # (1920, 50, 25)

    in_maps = []
    for k in range(NCORE):
        xw = wnd[b * k: b * (k + 1)]        # (240, 50, 25)
        xT = np.empty((26, Wn, b), np.float16)
        xT[:25] = xw.transpose(2, 1, 0).astype(np.float16)
        xT[25] = 1.0
        in_maps.append({"x_in": np.ascontiguousarray(xT.reshape(26, TOK)), **wmap})

    res = run_bass_kernel_spmd(nc, in_maps, list(range(NCORE)), trace=trace)
    if trace:
        kernel._last_exec_ns = res.exec_time_ns
        kernel._last_mean_exec_ns = res.mean_exec_time_ns
        kernel._last_res = res

    preds, c3ds, mus, lvs = [], [], [], []
    for k in range(NCORE):
        o = res.results[k]
        pk = o["pred"].reshape(25, Wn, b).transpose(2, 1, 0)   # (j, t, f)
        preds.append(pk.reshape(b // 15, 15 * Wn, F))
        ck = o["c3d"].reshape(C3D, Wn, b).transpose(2, 1, 0)
        c3ds.append(ck.reshape(b // 15, 15 * Wn, C3D))
        mus.append(o["muo"].reshape(128, 2, b).transpose(2, 1, 0).reshape(b, Z))
        lvs.append(o["lvo"].reshape(128, 2, b).transpose(2, 1, 0).reshape(b, Z))

    prediction = np.concatenate(preds, 0).astype(np.float32)
    coeff_3dmm = np.concatenate(c3ds, 0).astype(np.float32)
    mu = np.concatenate(mus, 0).astype(np.float32)
    logvar = np.concatenate(lvs, 0).astype(np.float32)
    return prediction, coeff_3dmm, mu, logvar
